# revision 27
# baseline (speedup 1.0000x reference)
"""Trainium2 Bass kernel for nn_AxispoolingMamba.

Sharding: 8 cores = (batch b in 0..3) x (h-half in 0..1).
Each core gets x0[b, :, half*128:(half+1)*128, :]  ([256c, 128h, 256w]).
  Stage A: partial mean over w  -> pair AllGather -> full x_h[b]
  model1_h (replicated within pair, b-sharded across pairs)
  Stage C: gate by xm_h (own h rows) + partial sum over h -> pair AllReduce
  model1_w
  Stage D: out = xm_w * x0  (own h rows) -> per-core output shard.

Layout convention on chip: channel dim on partitions (tiles of 128),
sequence dim l on the free axis.  Selective scan uses the DVE
tensor_tensor_scan instruction: state = aexp[t]*state + dBu[t].
"""

import sys

sys.path.insert(0, "/opt/trn_rl_repo")

from contextlib import ExitStack  # noqa: E402

import numpy as np  # noqa: E402

import concourse.bass as bass  # noqa: E402
import concourse.bacc as bacc  # noqa: E402
import concourse.mybir as mybir  # noqa: E402
import concourse.tile as tile  # noqa: E402

F32 = mybir.dt.float32
AF = mybir.ActivationFunctionType
OP = mybir.AluOpType

D_MODEL = 256
D_INNER = 512
D_STATE = 16
DT_RANK = 16
D_CONV = 4
DEPTH = 2
L = 256          # sequence length for both mamba passes (h or w)
HLOC = 128       # h rows owned by one core
NMT_IN = 2 * D_INNER // 128   # 8
NDT = D_INNER // 128          # 4
NCT = D_MODEL // 128          # 2


def _block(nc, tc, ctx, P, i, x):
    """One mamba block. x: sbuf tile [128, NCT, L] (c-major). Returns same shape."""
    ap = P["act"]
    sp = P["scan"]
    pp = P["psum"]

    W_in, W_xp, W_dt, W_out = P["W_in"][i], P["W_xp"][i], P["W_dt"][i], P["W_out"][i]
    cw, cb, dtb, nA, Dpar = P["cw"][i], P["cb"][i], P["dtb"][i], P["nA"][i], P["Dp"][i]
    ones1 = P["ones1"]

    # ---- in_proj: xr[1024, L] = in_w @ x ----
    xx = ap.tile([128, NDT, L + D_CONV - 1], F32, tag="xx")   # left-pad 3 for conv
    res = ap.tile([128, NDT, L], F32, tag="res")
    nc.vector.memset(xx[:, :, 0:D_CONV - 1], 0.0)
    for mt in range(NMT_IN):
        ps = pp.tile([128, L], F32, tag="ps")
        for ct in range(NCT):
            nc.tensor.matmul(ps[:], W_in[:, ct, mt * 128:(mt + 1) * 128],
                             x[:, ct, :], start=(ct == 0), stop=(ct == NCT - 1))
        if mt < NDT:
            nc.vector.tensor_copy(xx[:, mt, D_CONV - 1:], ps[:])
        else:
            nc.vector.tensor_copy(res[:, mt - NDT, :], ps[:])

    # ---- causal depthwise conv + bias + silu ----
    u = ap.tile([128, NDT, L], F32, tag="u")
    cacc = ap.tile([128, NDT, L], F32, tag="cacc")
    for dt in range(NDT):
        nc.vector.tensor_scalar_mul(cacc[:, dt, :], xx[:, dt, 0:L], cw[:, dt, 0:1])
        for j in range(1, D_CONV):
            nc.vector.scalar_tensor_tensor(cacc[:, dt, :], xx[:, dt, j:j + L],
                                           cw[:, dt, j:j + 1], cacc[:, dt, :],
                                           OP.mult, OP.add)
        nc.scalar.activation(u[:, dt, :], cacc[:, dt, :], AF.Silu,
                             bias=cb[:, dt, :], scale=1.0)

    # ---- x_dbl = xproj @ u : [48, L] ----
    ps2 = pp.tile([48, L], F32, tag="ps48")
    for dt in range(NDT):
        nc.tensor.matmul(ps2[:], W_xp[:, dt, :], u[:, dt, :],
                         start=(dt == 0), stop=(dt == NDT - 1))
    xdbl = ap.tile([48, L], F32, tag="xdbl")
    nc.vector.tensor_copy(xdbl[:], ps2[:])

    # ---- delta = softplus(dt_w @ delta_r + dt_b) : [512, L] ----
    delta = ap.tile([128, NDT, L], F32, tag="cacc")  # reuse cacc slot
    for dt in range(NDT):
        ps3 = pp.tile([128, L], F32, tag="ps")
        nc.tensor.matmul(ps3[:], W_dt[:, dt * 128:(dt + 1) * 128],
                         xdbl[0:DT_RANK, :], start=True, stop=True)
        # softplus(v) = ln(1 + exp(v)), v = raw + dt_b
        nc.scalar.activation(delta[:, dt, :], ps3[:], AF.Exp,
                             bias=dtb[:, dt, :], scale=1.0)
        nc.vector.tensor_scalar_add(delta[:, dt, :], delta[:, dt, :], 1.0)
        nc.scalar.activation(delta[:, dt, :], delta[:, dt, :], AF.Ln)

    # ---- broadcast B, C across partitions: [128, 16, L] ----
    # flatten [16, L] -> [1, 16*L] via DMA, then ones[1,128].T @ flat chunks
    Bc = ap.tile([128, D_STATE, L], F32, tag="Bc")
    Cc = ap.tile([128, D_STATE, L], F32, tag="Cc")
    for t, base in ((Bc, DT_RANK), (Cc, DT_RANK + D_STATE)):
        bc_flat = ap.tile([1, D_STATE * L], F32, tag="bcflat")
        nc.sync.dma_start(bc_flat[:], xdbl[base:base + D_STATE, :])
        for ch in range(D_STATE * L // 512):
            ps4 = pp.tile([128, 512], F32, tag="ps512")
            nc.tensor.matmul(ps4[:], ones1[:], bc_flat[0:1, ch * 512:(ch + 1) * 512],
                             start=True, stop=True)
            nc.vector.tensor_copy(
                t[:, 2 * ch:2 * ch + 2, :].rearrange("p n l -> p (n l)"), ps4[:])

    # ---- du = delta * u ----
    du = ap.tile([128, NDT, L], F32, tag="du")
    nc.vector.tensor_mul(du[:], delta[:], u[:])

    # ---- selective scan per d-tile ----
    y = ap.tile([128, NDT, L], F32, tag="y")
    for dt in range(NDT):
        aexp = sp.tile([128, D_STATE, L], F32, tag="aexp")
        dbu = sp.tile([128, D_STATE, L], F32, tag="dbu")
        hh = sp.tile([128, D_STATE, L], F32, tag="hh")
        for n in range(D_STATE):
            nc.scalar.activation(aexp[:, n, :], delta[:, dt, :], AF.Exp,
                                 scale=nA[:, dt, n:n + 1])
        nc.vector.tensor_mul(dbu[:], du[:, dt:dt + 1, :].broadcast_to([128, D_STATE, L]),
                             Bc[:])
        for n in range(D_STATE):
            nc.vector.tensor_tensor_scan(hh[:, n, :], aexp[:, n, :], dbu[:, n, :],
                                         0.0, OP.mult, OP.add)
        hc = aexp  # reuse buffer
        nc.vector.tensor_mul(hc[:], hh[:], Cc[:])
        nc.vector.tensor_reduce(y[:, dt, :], hc[:].rearrange("p n l -> p l n"),
                                axis=mybir.AxisListType.X, op=OP.add)

    # ---- y = (y + u*D) * silu(res); out_proj ----
    for dt in range(NDT):
        nc.vector.scalar_tensor_tensor(y[:, dt, :], u[:, dt, :], Dpar[:, dt, :],
                                       y[:, dt, :], OP.mult, OP.add)
    nc.scalar.activation(res[:], res[:], AF.Silu)
    nc.vector.tensor_mul(y[:], y[:], res[:])

    xo = ap.tile([128, NCT, L], F32, tag="xo")
    for mt in range(NCT):
        ps5 = pp.tile([128, L], F32, tag="ps")
        for dt in range(NDT):
            nc.tensor.matmul(ps5[:], W_out[:, dt, mt * 128:(mt + 1) * 128],
                             y[:, dt, :], start=(dt == 0), stop=(dt == NDT - 1))
        nc.vector.tensor_copy(xo[:, mt, :], ps5[:])
    return xo


def _model1(nc, tc, ctx, P, x):
    for i in range(DEPTH):
        x = _block(nc, tc, ctx, P, i, x)
    return x


HCH = 8           # h rows per streaming chunk
NHC = HLOC // HCH  # 16 chunks


def build(n_cores=8, fake_pair=False):
    nc = bacc.Bacc(None, target_bir_lowering=False)
    nc.num_devices = n_cores

    x0s = nc.dram_tensor("x0s", [D_MODEL, HLOC, 256], F32, kind="ExternalInput")
    w_in = nc.dram_tensor("w_in_t", [DEPTH, D_MODEL, 2 * D_INNER], F32, kind="ExternalInput")
    w_xp = nc.dram_tensor("w_xp_t", [DEPTH, D_INNER, 48], F32, kind="ExternalInput")
    w_dt = nc.dram_tensor("w_dt_t", [DEPTH, DT_RANK, D_INNER], F32, kind="ExternalInput")
    w_out = nc.dram_tensor("w_out_t", [DEPTH, D_INNER, D_MODEL], F32, kind="ExternalInput")
    cw_d = nc.dram_tensor("conv_w_r", [DEPTH, D_INNER, D_CONV], F32, kind="ExternalInput")
    cb_d = nc.dram_tensor("conv_b", [DEPTH, D_INNER], F32, kind="ExternalInput")
    dtb_d = nc.dram_tensor("dt_b", [DEPTH, D_INNER], F32, kind="ExternalInput")
    nA_d = nc.dram_tensor("neg_a", [DEPTH, D_INNER, D_STATE], F32, kind="ExternalInput")
    Dp_d = nc.dram_tensor("d_par", [DEPTH, D_INNER], F32, kind="ExternalInput")
    hsel_d = nc.dram_tensor("hsel", [128, 2], F32, kind="ExternalInput")
    out_d = nc.dram_tensor("out", [D_MODEL, HLOC, 256], F32, kind="ExternalOutput")

    with tile.TileContext(nc) as tc, ExitStack() as ctx:
        wp = ctx.enter_context(tc.tile_pool(name="weights", bufs=1))
        ap = ctx.enter_context(tc.tile_pool(name="act", bufs=1))
        sp = ctx.enter_context(tc.tile_pool(name="scan", bufs=1))
        stp = ctx.enter_context(tc.tile_pool(name="stage", bufs=3))
        stpo = ctx.enter_context(tc.tile_pool(name="stageout", bufs=2))
        pp = ctx.enter_context(tc.tile_pool(name="psum", bufs=2, space="PSUM"))
        dp = ctx.enter_context(tc.tile_pool(name="dram", bufs=1, space="DRAM"))

        P = {"act": ap, "scan": sp, "psum": pp,
             "W_in": [], "W_xp": [], "W_dt": [], "W_out": [],
             "cw": [], "cb": [], "dtb": [], "nA": [], "Dp": []}
        for i in range(DEPTH):
            wi = wp.tile([128, NCT, 2 * D_INNER], F32, tag=f"win{i}")
            for ct in range(NCT):
                nc.sync.dma_start(wi[:, ct, :], w_in[i, ct * 128:(ct + 1) * 128, :])
            P["W_in"].append(wi)
            wx = wp.tile([128, NDT, 48], F32, tag=f"wxp{i}")
            wo = wp.tile([128, NDT, D_MODEL], F32, tag=f"wout{i}")
            cwt = wp.tile([128, NDT, D_CONV], F32, tag=f"cw{i}")
            cbt = wp.tile([128, NDT, 1], F32, tag=f"cb{i}")
            dtbt = wp.tile([128, NDT, 1], F32, tag=f"dtb{i}")
            nAt = wp.tile([128, NDT, D_STATE], F32, tag=f"na{i}")
            dpt = wp.tile([128, NDT, 1], F32, tag=f"dp{i}")
            for dt in range(NDT):
                sl = slice(dt * 128, (dt + 1) * 128)
                nc.sync.dma_start(wx[:, dt, :], w_xp[i, sl, :])
                nc.sync.dma_start(wo[:, dt, :], w_out[i, sl, :])
                nc.sync.dma_start(cwt[:, dt, :], cw_d[i, sl, :])
                nc.sync.dma_start(cbt[:, dt, :], cb_d[i, sl][:, None])
                nc.sync.dma_start(dtbt[:, dt, :], dtb_d[i, sl][:, None])
                nc.sync.dma_start(nAt[:, dt, :], nA_d[i, sl, :])
                nc.sync.dma_start(dpt[:, dt, :], Dp_d[i, sl][:, None])
            wd = wp.tile([DT_RANK, D_INNER], F32, tag=f"wdt{i}")
            nc.sync.dma_start(wd[:], w_dt[i])
            P["W_xp"].append(wx); P["W_out"].append(wo); P["W_dt"].append(wd)
            P["cw"].append(cwt); P["cb"].append(cbt); P["dtb"].append(dtbt)
            P["nA"].append(nAt); P["Dp"].append(dpt)
        ones1 = wp.tile([1, 128], F32, tag="ones1")
        nc.vector.memset(ones1[:], 1.0)
        P["ones1"] = ones1
        hsel = wp.tile([128, 2], F32, tag="hsel")
        nc.sync.dma_start(hsel[:], hsel_d[:])

        # ================= Stage A: partial sum over w =================
        xh_part = ap.tile([128, NCT, HLOC], F32, tag="xh_part")
        for ct in range(NCT):
            for hc in range(NHC):
                t = stp.tile([128, HCH, 256], F32, tag="x0chunk")
                nc.sync.dma_start(t[:], x0s[ct * 128:(ct + 1) * 128,
                                             hc * HCH:(hc + 1) * HCH, :])
                nc.vector.tensor_reduce(xh_part[:, ct, hc * HCH:(hc + 1) * HCH],
                                        t[:], axis=mybir.AxisListType.X, op=OP.add)

        # ================= Exchange 1: pair AllGather =================
        xh_full = ap.tile([128, NCT, L], F32, tag="xh_full")
        gin = dp.tile([128, NCT, HLOC], F32)
        gout = dp.tile([2, 128, NCT, HLOC], F32)
        nc.sync.dma_start(gin[:], xh_part[:])
        if fake_pair:
            nc.sync.dma_start(gout[0], gin[:])
            nc.sync.dma_start(gout[1], gin[:])
        else:
            groups = [[2 * b, 2 * b + 1] for b in range(n_cores // 2)]
            nc.gpsimd.collective_compute(
                "AllGather", OP.bypass, replica_groups=groups,
                ins=[gin.opt()], outs=[gout.opt()])
        for ct in range(NCT):
            for half in range(2):
                nc.sync.dma_start(xh_full[:, ct, half * HLOC:(half + 1) * HLOC],
                                  gout[half, :, ct, :])

        # ================= model1 over h =================
        xmh = _model1(nc, tc, ctx, P, xh_full)

        # gate rows for my h-half: gate[c, hloc] (select half via hsel one-hot)
        gate = ap.tile([128, NCT, HLOC], F32, tag="gate")
        for ct in range(NCT):
            nc.vector.tensor_scalar_mul(gate[:, ct, :], xmh[:, ct, 0:HLOC],
                                        hsel[:, 0:1])
            nc.vector.scalar_tensor_tensor(gate[:, ct, :], xmh[:, ct, HLOC:],
                                           hsel[:, 1:2], gate[:, ct, :],
                                           OP.mult, OP.add)

        # ================= Stage C: gated partial sum over h =================
        xw_part = ap.tile([128, NCT, 256], F32, tag="xw_part")
        for ct in range(NCT):
            for hc in range(NHC):
                t = stp.tile([128, HCH, 256], F32, tag="x0chunk")
                nc.sync.dma_start(t[:], x0s[ct * 128:(ct + 1) * 128,
                                             hc * HCH:(hc + 1) * HCH, :])
                for hi in range(HCH):
                    h = hc * HCH + hi
                    if h == 0:
                        nc.vector.tensor_scalar_mul(xw_part[:, ct, :], t[:, hi, :],
                                                    gate[:, ct, h:h + 1])
                    else:
                        nc.vector.scalar_tensor_tensor(xw_part[:, ct, :], t[:, hi, :],
                                                       gate[:, ct, h:h + 1],
                                                       xw_part[:, ct, :],
                                                       OP.mult, OP.add)

        # ================= Exchange 2: pair AllReduce =================
        xw = ap.tile([128, NCT, 256], F32, tag="xw")
        rin = dp.tile([128, NCT, 256], F32)
        rout = dp.tile([128, NCT, 256], F32)
        nc.sync.dma_start(rin[:], xw_part[:])
        if fake_pair:
            nc.sync.dma_start(rout[:], rin[:])
        else:
            groups = [[2 * b, 2 * b + 1] for b in range(n_cores // 2)]
            nc.gpsimd.collective_compute(
                "AllReduce", OP.add, replica_groups=groups,
                ins=[rin.opt()], outs=[rout.opt()])
        nc.sync.dma_start(xw[:], rout[:])

        # ================= model1 over w =================
        xmw = _model1(nc, tc, ctx, P, xw)

        # ================= Stage D: out = xmw (bcast over h) * x0 =================
        for ct in range(NCT):
            for hc in range(NHC):
                t = stp.tile([128, HCH, 256], F32, tag="x0chunk")
                o = stpo.tile([128, HCH, 256], F32, tag="ochunk")
                nc.sync.dma_start(t[:], x0s[ct * 128:(ct + 1) * 128,
                                             hc * HCH:(hc + 1) * HCH, :])
                nc.vector.tensor_mul(
                    o[:], t[:],
                    xmw[:, ct:ct + 1, :].broadcast_to([128, HCH, 256]))
                nc.sync.dma_start(out_d[ct * 128:(ct + 1) * 128,
                                        hc * HCH:(hc + 1) * HCH, :], o[:])

    nc.compile()
    return nc


def _prep_host(inputs):
    x0 = np.ascontiguousarray(inputs["x0"], dtype=np.float32)
    in_w = np.asarray(inputs["in_w"], np.float32)
    conv_w = np.asarray(inputs["conv_w"], np.float32)
    conv_b = np.asarray(inputs["conv_b"], np.float32)
    xproj_w = np.asarray(inputs["xproj_w"], np.float32)
    dt_w = np.asarray(inputs["dt_w"], np.float32)
    dt_b = np.asarray(inputs["dt_b"], np.float32)
    A_log = np.asarray(inputs["A_log"], np.float32)
    Dp = np.asarray(inputs["Dp"], np.float32)
    out_w = np.asarray(inputs["out_w"], np.float32)

    w = {}
    # fold the 1/256 pooling mean (exact power of two) into depth-0 in_proj
    w_in_t = np.ascontiguousarray(in_w.transpose(0, 2, 1))
    w_in_t[0] = w_in_t[0] * np.float32(2.0 ** -8)
    w["w_in_t"] = w_in_t
    w["w_xp_t"] = np.ascontiguousarray(xproj_w.transpose(0, 2, 1))
    w["w_dt_t"] = np.ascontiguousarray(dt_w.transpose(0, 2, 1))
    w["w_out_t"] = np.ascontiguousarray(out_w.transpose(0, 2, 1))
    w["conv_w_r"] = np.ascontiguousarray(conv_w[:, :, 0, :])
    w["conv_b"] = conv_b
    w["dt_b"] = dt_b
    w["neg_a"] = -np.exp(A_log)
    w["d_par"] = Dp
    return x0, w


def kernel(**inputs):
    from concourse.bass_utils import run_bass_kernel_spmd

    x0, w = _prep_host(inputs)
    nc = build(n_cores=8)

    in_maps = []
    for k in range(8):
        b, half = k // 2, k % 2
        m = dict(w)
        m["x0s"] = np.ascontiguousarray(x0[b, :, half * 128:(half + 1) * 128, :])
        hs = np.zeros((128, 2), np.float32)
        hs[:, half] = 1.0
        m["hsel"] = hs
        in_maps.append(m)

    res = run_bass_kernel_spmd(nc, in_maps, core_ids=list(range(8)))
    out = np.empty((4, 256, 256, 256), np.float32)
    for k in range(8):
        b, half = k // 2, k % 2
        out[b, :, half * 128:(half + 1) * 128, :] = res.results[k]["out"]
    return out


# revision 29
# speedup vs baseline: 1.0324x; 1.0324x over previous
"""Trainium2 Bass kernel for nn_AxispoolingMamba.

Sharding: 8 cores = (batch b in 0..3) x (h-half in 0..1).
Each core gets x0[b, :, half*128:(half+1)*128, :]  ([256c, 128h, 256w]).
  Stage A: partial mean over w  -> pair AllGather -> full x_h[b]
  model1_h (replicated within pair, b-sharded across pairs)
  Stage C: gate by xm_h (own h rows) + partial sum over h -> pair AllReduce
  model1_w
  Stage D: out = xm_w * x0  (own h rows) -> per-core output shard.

Layout convention on chip: channel dim on partitions (tiles of 128),
sequence dim l on the free axis.  Selective scan uses the DVE
tensor_tensor_scan instruction: state = aexp[t]*state + dBu[t].
"""

import sys

sys.path.insert(0, "/opt/trn_rl_repo")

from contextlib import ExitStack  # noqa: E402

import numpy as np  # noqa: E402

import concourse.bass as bass  # noqa: E402
import concourse.bacc as bacc  # noqa: E402
import concourse.mybir as mybir  # noqa: E402
import concourse.tile as tile  # noqa: E402

F32 = mybir.dt.float32
AF = mybir.ActivationFunctionType
OP = mybir.AluOpType

D_MODEL = 256
D_INNER = 512
D_STATE = 16
DT_RANK = 16
D_CONV = 4
DEPTH = 2
L = 256          # sequence length for both mamba passes (h or w)
HLOC = 128       # h rows owned by one core
NMT_IN = 2 * D_INNER // 128   # 8
NDT = D_INNER // 128          # 4
NCT = D_MODEL // 128          # 2


def _block(nc, tc, ctx, P, i, x):
    """One mamba block. x: sbuf tile [128, NCT, L] (c-major). Returns same shape."""
    ap = P["act"]
    sp = P["scan"]
    pp = P["psum"]

    W_in, W_xp, W_dt, W_out = P["W_in"][i], P["W_xp"][i], P["W_dt"][i], P["W_out"][i]
    cw, cb, dtb, nA, Dpar = P["cw"][i], P["cb"][i], P["dtb"][i], P["nA"][i], P["Dp"][i]
    ones1 = P["ones1"]

    # ---- in_proj: xr[1024, L] = in_w @ x ----
    xx = ap.tile([128, NDT, L + D_CONV - 1], F32, tag="xx")   # left-pad 3 for conv
    res = ap.tile([128, NDT, L], F32, tag="res")
    nc.vector.memset(xx[:, :, 0:D_CONV - 1], 0.0)
    for mt in range(NMT_IN):
        ps = pp.tile([128, L], F32, tag="ps")
        for ct in range(NCT):
            nc.tensor.matmul(ps[:], W_in[:, ct, mt * 128:(mt + 1) * 128],
                             x[:, ct, :], start=(ct == 0), stop=(ct == NCT - 1))
        if mt < NDT:
            nc.scalar.activation(xx[:, mt, D_CONV - 1:], ps[:], AF.Copy)
        else:
            nc.scalar.activation(res[:, mt - NDT, :], ps[:], AF.Copy)

    # ---- causal depthwise conv + bias + silu ----
    u = ap.tile([128, NDT, L], F32, tag="u")
    cacc = ap.tile([128, NDT, L], F32, tag="cacc")
    for dt in range(NDT):
        nc.vector.tensor_scalar_mul(cacc[:, dt, :], xx[:, dt, 0:L], cw[:, dt, 0:1])
        for j in range(1, D_CONV):
            nc.vector.scalar_tensor_tensor(cacc[:, dt, :], xx[:, dt, j:j + L],
                                           cw[:, dt, j:j + 1], cacc[:, dt, :],
                                           OP.mult, OP.add)
        nc.scalar.activation(u[:, dt, :], cacc[:, dt, :], AF.Silu,
                             bias=cb[:, dt, :], scale=1.0)

    # ---- x_dbl = xproj @ u : [48, L] ----
    ps2 = pp.tile([48, L], F32, tag="ps48")
    for dt in range(NDT):
        nc.tensor.matmul(ps2[:], W_xp[:, dt, :], u[:, dt, :],
                         start=(dt == 0), stop=(dt == NDT - 1))
    xdbl = ap.tile([48, L], F32, tag="xdbl")
    nc.vector.tensor_copy(xdbl[:], ps2[:])

    # ---- delta = softplus(dt_w @ delta_r + dt_b) : [512, L] ----
    delta = ap.tile([128, NDT, L], F32, tag="cacc")  # reuse cacc slot
    for dt in range(NDT):
        ps3 = pp.tile([128, L], F32, tag="ps")
        nc.tensor.matmul(ps3[:], W_dt[:, dt * 128:(dt + 1) * 128],
                         xdbl[0:DT_RANK, :], start=True, stop=True)
        # softplus(v) = ln(1 + exp(v)), v = raw + dt_b
        nc.scalar.activation(delta[:, dt, :], ps3[:], AF.Exp,
                             bias=dtb[:, dt, :], scale=1.0)
        nc.vector.tensor_scalar_add(delta[:, dt, :], delta[:, dt, :], 1.0)
        nc.scalar.activation(delta[:, dt, :], delta[:, dt, :], AF.Ln)

    # ---- broadcast B, C across partitions: [128, 16, L] ----
    # flatten [16, L] -> [1, 16*L] via DMA, then ones[1,128].T @ flat chunks
    Bc = ap.tile([128, D_STATE, L], F32, tag="Bc")
    Cc = ap.tile([128, D_STATE, L], F32, tag="Cc")
    for t, base in ((Bc, DT_RANK), (Cc, DT_RANK + D_STATE)):
        bc_flat = ap.tile([1, D_STATE * L], F32, tag="bcflat")
        nc.sync.dma_start(bc_flat[:], xdbl[base:base + D_STATE, :])
        for ch in range(D_STATE * L // 512):
            ps4 = pp.tile([128, 512], F32, tag="ps512")
            nc.tensor.matmul(ps4[:], ones1[:], bc_flat[0:1, ch * 512:(ch + 1) * 512],
                             start=True, stop=True)
            nc.scalar.activation(
                t[:, 2 * ch:2 * ch + 2, :].rearrange("p n l -> p (n l)"), ps4[:],
                AF.Copy)

    # ---- du = delta * u ----
    du = ap.tile([128, NDT, L], F32, tag="du")
    nc.vector.tensor_mul(du[:], delta[:], u[:])

    # ---- selective scan per d-tile ----
    y = ap.tile([128, NDT, L], F32, tag="y")
    for dt in range(NDT):
        aexp = sp.tile([128, D_STATE, L], F32, tag="aexp")
        dbu = sp.tile([128, D_STATE, L], F32, tag="dbu")
        hh = sp.tile([128, D_STATE, L], F32, tag="hh")
        for n in range(D_STATE):
            nc.scalar.activation(aexp[:, n, :], delta[:, dt, :], AF.Exp,
                                 scale=nA[:, dt, n:n + 1])
        nc.vector.tensor_mul(dbu[:], du[:, dt:dt + 1, :].broadcast_to([128, D_STATE, L]),
                             Bc[:])
        for n in range(D_STATE):
            nc.vector.tensor_tensor_scan(hh[:, n, :], aexp[:, n, :], dbu[:, n, :],
                                         0.0, OP.mult, OP.add)
        hc = aexp  # reuse buffer
        nc.vector.tensor_mul(hc[:], hh[:], Cc[:])
        nc.vector.tensor_reduce(y[:, dt, :], hc[:].rearrange("p n l -> p l n"),
                                axis=mybir.AxisListType.X, op=OP.add)

    # ---- y = (y + u*D) * silu(res); out_proj ----
    for dt in range(NDT):
        nc.vector.scalar_tensor_tensor(y[:, dt, :], u[:, dt, :], Dpar[:, dt, :],
                                       y[:, dt, :], OP.mult, OP.add)
    nc.scalar.activation(res[:], res[:], AF.Silu)
    nc.vector.tensor_mul(y[:], y[:], res[:])

    xo = ap.tile([128, NCT, L], F32, tag="xo")
    for mt in range(NCT):
        ps5 = pp.tile([128, L], F32, tag="ps")
        for dt in range(NDT):
            nc.tensor.matmul(ps5[:], W_out[:, dt, mt * 128:(mt + 1) * 128],
                             y[:, dt, :], start=(dt == 0), stop=(dt == NDT - 1))
        nc.vector.tensor_copy(xo[:, mt, :], ps5[:])
    return xo


def _model1(nc, tc, ctx, P, x):
    for i in range(DEPTH):
        x = _block(nc, tc, ctx, P, i, x)
    return x


HCH = 8           # h rows per streaming chunk
NHC = HLOC // HCH  # 16 chunks


def build(n_cores=8, fake_pair=False):
    nc = bacc.Bacc(None, target_bir_lowering=False)
    nc.num_devices = n_cores

    x0s = nc.dram_tensor("x0s", [D_MODEL, HLOC, 256], F32, kind="ExternalInput")
    w_in = nc.dram_tensor("w_in_t", [DEPTH, D_MODEL, 2 * D_INNER], F32, kind="ExternalInput")
    w_xp = nc.dram_tensor("w_xp_t", [DEPTH, D_INNER, 48], F32, kind="ExternalInput")
    w_dt = nc.dram_tensor("w_dt_t", [DEPTH, DT_RANK, D_INNER], F32, kind="ExternalInput")
    w_out = nc.dram_tensor("w_out_t", [DEPTH, D_INNER, D_MODEL], F32, kind="ExternalInput")
    cw_d = nc.dram_tensor("conv_w_r", [DEPTH, D_INNER, D_CONV], F32, kind="ExternalInput")
    cb_d = nc.dram_tensor("conv_b", [DEPTH, D_INNER], F32, kind="ExternalInput")
    dtb_d = nc.dram_tensor("dt_b", [DEPTH, D_INNER], F32, kind="ExternalInput")
    nA_d = nc.dram_tensor("neg_a", [DEPTH, D_INNER, D_STATE], F32, kind="ExternalInput")
    Dp_d = nc.dram_tensor("d_par", [DEPTH, D_INNER], F32, kind="ExternalInput")
    hsel_d = nc.dram_tensor("hsel", [128, 2], F32, kind="ExternalInput")
    out_d = nc.dram_tensor("out", [D_MODEL, HLOC, 256], F32, kind="ExternalOutput")

    with tile.TileContext(nc) as tc, ExitStack() as ctx:
        wp = ctx.enter_context(tc.tile_pool(name="weights", bufs=1))
        ap = ctx.enter_context(tc.tile_pool(name="act", bufs=1))
        sp = ctx.enter_context(tc.tile_pool(name="scan", bufs=1))
        stp = ctx.enter_context(tc.tile_pool(name="stage", bufs=3))
        stpo = ctx.enter_context(tc.tile_pool(name="stageout", bufs=2))
        pp = ctx.enter_context(tc.tile_pool(name="psum", bufs=2, space="PSUM"))
        dp = ctx.enter_context(tc.tile_pool(name="dram", bufs=1, space="DRAM"))

        P = {"act": ap, "scan": sp, "psum": pp,
             "W_in": [], "W_xp": [], "W_dt": [], "W_out": [],
             "cw": [], "cb": [], "dtb": [], "nA": [], "Dp": []}
        for i in range(DEPTH):
            wi = wp.tile([128, NCT, 2 * D_INNER], F32, tag=f"win{i}")
            for ct in range(NCT):
                nc.sync.dma_start(wi[:, ct, :], w_in[i, ct * 128:(ct + 1) * 128, :])
            P["W_in"].append(wi)
            wx = wp.tile([128, NDT, 48], F32, tag=f"wxp{i}")
            wo = wp.tile([128, NDT, D_MODEL], F32, tag=f"wout{i}")
            cwt = wp.tile([128, NDT, D_CONV], F32, tag=f"cw{i}")
            cbt = wp.tile([128, NDT, 1], F32, tag=f"cb{i}")
            dtbt = wp.tile([128, NDT, 1], F32, tag=f"dtb{i}")
            nAt = wp.tile([128, NDT, D_STATE], F32, tag=f"na{i}")
            dpt = wp.tile([128, NDT, 1], F32, tag=f"dp{i}")
            for dt in range(NDT):
                sl = slice(dt * 128, (dt + 1) * 128)
                nc.sync.dma_start(wx[:, dt, :], w_xp[i, sl, :])
                nc.sync.dma_start(wo[:, dt, :], w_out[i, sl, :])
                nc.sync.dma_start(cwt[:, dt, :], cw_d[i, sl, :])
                nc.sync.dma_start(cbt[:, dt, :], cb_d[i, sl][:, None])
                nc.sync.dma_start(dtbt[:, dt, :], dtb_d[i, sl][:, None])
                nc.sync.dma_start(nAt[:, dt, :], nA_d[i, sl, :])
                nc.sync.dma_start(dpt[:, dt, :], Dp_d[i, sl][:, None])
            wd = wp.tile([DT_RANK, D_INNER], F32, tag=f"wdt{i}")
            nc.sync.dma_start(wd[:], w_dt[i])
            P["W_xp"].append(wx); P["W_out"].append(wo); P["W_dt"].append(wd)
            P["cw"].append(cwt); P["cb"].append(cbt); P["dtb"].append(dtbt)
            P["nA"].append(nAt); P["Dp"].append(dpt)
        ones1 = wp.tile([1, 128], F32, tag="ones1")
        nc.vector.memset(ones1[:], 1.0)
        P["ones1"] = ones1
        hsel = wp.tile([128, 2], F32, tag="hsel")
        nc.sync.dma_start(hsel[:], hsel_d[:])

        # ================= Stage A: partial sum over w =================
        xh_part = ap.tile([128, NCT, HLOC], F32, tag="xh_part")
        for ct in range(NCT):
            for hc in range(NHC):
                t = stp.tile([128, HCH, 256], F32, tag="x0chunk")
                nc.sync.dma_start(t[:], x0s[ct * 128:(ct + 1) * 128,
                                             hc * HCH:(hc + 1) * HCH, :])
                nc.vector.tensor_reduce(xh_part[:, ct, hc * HCH:(hc + 1) * HCH],
                                        t[:], axis=mybir.AxisListType.X, op=OP.add)

        # ================= Exchange 1: pair AllGather =================
        xh_full = ap.tile([128, NCT, L], F32, tag="xh_full")
        gin = dp.tile([128, NCT, HLOC], F32)
        gout = dp.tile([2, 128, NCT, HLOC], F32)
        nc.sync.dma_start(gin[:], xh_part[:])
        if fake_pair:
            nc.sync.dma_start(gout[0], gin[:])
            nc.sync.dma_start(gout[1], gin[:])
        else:
            groups = [[2 * b, 2 * b + 1] for b in range(n_cores // 2)]
            nc.gpsimd.collective_compute(
                "AllGather", OP.bypass, replica_groups=groups,
                ins=[gin.opt()], outs=[gout.opt()])
        for ct in range(NCT):
            for half in range(2):
                nc.sync.dma_start(xh_full[:, ct, half * HLOC:(half + 1) * HLOC],
                                  gout[half, :, ct, :])

        # ================= model1 over h =================
        xmh = _model1(nc, tc, ctx, P, xh_full)

        # gate rows for my h-half: gate[c, hloc] (select half via hsel one-hot)
        gate = ap.tile([128, NCT, HLOC], F32, tag="gate")
        for ct in range(NCT):
            nc.vector.tensor_scalar_mul(gate[:, ct, :], xmh[:, ct, 0:HLOC],
                                        hsel[:, 0:1])
            nc.vector.scalar_tensor_tensor(gate[:, ct, :], xmh[:, ct, HLOC:],
                                           hsel[:, 1:2], gate[:, ct, :],
                                           OP.mult, OP.add)

        # ================= Stage C: gated partial sum over h =================
        xw_part = ap.tile([128, NCT, 256], F32, tag="xw_part")
        for ct in range(NCT):
            for hc in range(NHC):
                t = stp.tile([128, HCH, 256], F32, tag="x0chunk")
                nc.sync.dma_start(t[:], x0s[ct * 128:(ct + 1) * 128,
                                             hc * HCH:(hc + 1) * HCH, :])
                for hi in range(HCH):
                    h = hc * HCH + hi
                    if h == 0:
                        nc.vector.tensor_scalar_mul(xw_part[:, ct, :], t[:, hi, :],
                                                    gate[:, ct, h:h + 1])
                    else:
                        nc.vector.scalar_tensor_tensor(xw_part[:, ct, :], t[:, hi, :],
                                                       gate[:, ct, h:h + 1],
                                                       xw_part[:, ct, :],
                                                       OP.mult, OP.add)

        # ================= Exchange 2: pair AllReduce =================
        xw = ap.tile([128, NCT, 256], F32, tag="xw")
        rin = dp.tile([128, NCT, 256], F32)
        rout = dp.tile([128, NCT, 256], F32)
        nc.sync.dma_start(rin[:], xw_part[:])
        if fake_pair:
            nc.sync.dma_start(rout[:], rin[:])
        else:
            groups = [[2 * b, 2 * b + 1] for b in range(n_cores // 2)]
            nc.gpsimd.collective_compute(
                "AllReduce", OP.add, replica_groups=groups,
                ins=[rin.opt()], outs=[rout.opt()])
        nc.sync.dma_start(xw[:], rout[:])

        # ================= model1 over w =================
        xmw = _model1(nc, tc, ctx, P, xw)

        # ================= Stage D: out = xmw (bcast over h) * x0 =================
        for ct in range(NCT):
            for hc in range(NHC):
                t = stp.tile([128, HCH, 256], F32, tag="x0chunk")
                o = stpo.tile([128, HCH, 256], F32, tag="ochunk")
                nc.sync.dma_start(t[:], x0s[ct * 128:(ct + 1) * 128,
                                             hc * HCH:(hc + 1) * HCH, :])
                nc.vector.tensor_mul(
                    o[:], t[:],
                    xmw[:, ct:ct + 1, :].broadcast_to([128, HCH, 256]))
                nc.sync.dma_start(out_d[ct * 128:(ct + 1) * 128,
                                        hc * HCH:(hc + 1) * HCH, :], o[:])

    nc.compile()
    return nc


def _prep_host(inputs):
    x0 = np.ascontiguousarray(inputs["x0"], dtype=np.float32)
    in_w = np.asarray(inputs["in_w"], np.float32)
    conv_w = np.asarray(inputs["conv_w"], np.float32)
    conv_b = np.asarray(inputs["conv_b"], np.float32)
    xproj_w = np.asarray(inputs["xproj_w"], np.float32)
    dt_w = np.asarray(inputs["dt_w"], np.float32)
    dt_b = np.asarray(inputs["dt_b"], np.float32)
    A_log = np.asarray(inputs["A_log"], np.float32)
    Dp = np.asarray(inputs["Dp"], np.float32)
    out_w = np.asarray(inputs["out_w"], np.float32)

    w = {}
    # fold the 1/256 pooling mean (exact power of two) into depth-0 in_proj
    w_in_t = np.ascontiguousarray(in_w.transpose(0, 2, 1))
    w_in_t[0] = w_in_t[0] * np.float32(2.0 ** -8)
    w["w_in_t"] = w_in_t
    w["w_xp_t"] = np.ascontiguousarray(xproj_w.transpose(0, 2, 1))
    w["w_dt_t"] = np.ascontiguousarray(dt_w.transpose(0, 2, 1))
    w["w_out_t"] = np.ascontiguousarray(out_w.transpose(0, 2, 1))
    w["conv_w_r"] = np.ascontiguousarray(conv_w[:, :, 0, :])
    w["conv_b"] = conv_b
    w["dt_b"] = dt_b
    w["neg_a"] = -np.exp(A_log)
    w["d_par"] = Dp
    return x0, w


def kernel(**inputs):
    from concourse.bass_utils import run_bass_kernel_spmd

    x0, w = _prep_host(inputs)
    nc = build(n_cores=8)

    in_maps = []
    for k in range(8):
        b, half = k // 2, k % 2
        m = dict(w)
        m["x0s"] = np.ascontiguousarray(x0[b, :, half * 128:(half + 1) * 128, :])
        hs = np.zeros((128, 2), np.float32)
        hs[:, half] = 1.0
        m["hsel"] = hs
        in_maps.append(m)

    res = run_bass_kernel_spmd(nc, in_maps, core_ids=list(range(8)))
    out = np.empty((4, 256, 256, 256), np.float32)
    for k in range(8):
        b, half = k // 2, k % 2
        out[b, :, half * 128:(half + 1) * 128, :] = res.results[k]["out"]
    return out
